# revision 1
# baseline (speedup 1.0000x reference)
"""MixGARCH Trainium2 kernel, v3: B=8 block-scan, single-pass phase 2.

Same math as v2 (see kernel2.py), but the carry term is folded into the
phase-2 matmul contraction instead of a second matmul pass:

  - per-half rhs tile [128, NB]: rows 0..64 hold packed x^2 (8i+l layout),
    rows 64..128 hold S_{b-1} (carry state), placed there by an SBUF->SBUF
    DMA copy of the scan output (partition shift). lhsT_p[64+k, (j,kk)] =
    delta_k Wh^{j+1} injects the carry in the same matmul.
  - phase-1 d matmuls land in PSUM then are staged to SBUF f32 so the PE
    never waits on the scan chain.
  - phase-2 emitted c-major (all 8 (h,g) regions per chunk) so the PE hits
    steady state immediately after the d matmuls.
"""

import numpy as np
import ml_dtypes

BF16 = ml_dtypes.bfloat16

T = 524288
K = 64
NJ = 8
NCORES = 8
W = 256               # warmup steps per half
HALF = 32768
TT = W + HALF         # 33024 steps per half
B = 8                 # block size
NB = TT // B          # 4128 blocks per half
SC = 512              # phase-1 d chunk (blocks)
SCW = 1024            # scan chunk (blocks)
PC = 1024             # phase-2 chunk (blocks)
PCHUNKS = [(0, 1024), (1024, 2048), (2048, 3072), (3072, 4096), (4096, 4128)]
SCAN_ENGINE = "vector"   # or "gpsimd"
POOL_COPIES = False      # GpSimd cannot read PSUM (runtime error)

_CACHE = {}


def _weights_host(vars0, bias, Wx, Wh):
    Wx = Wx.astype(np.float64)
    Wh = Wh.astype(np.float64)
    bias = bias.astype(np.float64)
    s_steady = (bias + 1e-6) / (1.0 - Wh)
    whp = Wh[None, :] ** np.arange(10)[:, None]   # whp[e, k]

    # d/S layout partition: 64h + k.  xin rows (per half tile): 8i + l.
    # phase-2 out partition: 16j + kk (k = 16g + kk).
    lhsT_dh = np.zeros((64, 64), np.float64)
    for i in range(B):
        for l in range(NJ):
            for k in range(K):
                lhsT_dh[8 * i + l, k] = whp[7 - i, k] * Wx[k, l]

    # lhsT_p per g: [128, 128]; rows 0..64 intra-block, rows 64..128 carry.
    lhsT_p = np.zeros((128, 4 * 128), np.float64)
    for g in range(4):
        for j in range(B):
            for kk in range(16):
                k = 16 * g + kk
                col = 128 * g + 16 * j + kk
                for i in range(j + 1):
                    for l in range(NJ):
                        lhsT_p[8 * i + l, col] = whp[j - i, k] * Wx[k, l]
                lhsT_p[64 + k, col] = whp[j + 1, k]

    whB_rep = np.zeros((128, 1), np.float64)
    for h in range(2):
        for k in range(K):
            whB_rep[64 * h + k, :] = whp[8, k]

    bias_sb = np.zeros((128, 4), np.float64)
    for g in range(4):
        for j in range(B):
            for kk in range(16):
                bias_sb[16 * j + kk, g] = s_steady[16 * g + kk]

    cb = np.zeros((128, 576), np.float64)
    cb[:, 0:512] = lhsT_p
    cb[0:64, 512:576] = lhsT_dh
    cf = np.zeros((128, 5), np.float64)
    cf[:, 0:4] = bias_sb
    cf[:, 4:5] = whB_rep

    # initial S column (S layout 64h+k): core0 half A = vars0 - s, else 0
    winit = np.zeros((128, 1), np.float64)
    winit[0:64, 0] = vars0.astype(np.float64) - s_steady

    return {
        "constb": cb.astype(BF16),
        "constf": cf.astype(np.float32),
        "winit": winit.astype(BF16),
    }


def _pack_half(x2, core, h):
    start = core * 65536 + h * HALF
    if core == 0 and h == 0:
        rows = x2[0:TT]
    else:
        rows = x2[start - W:start + HALF]
    return rows.reshape(NB, B, NJ).transpose(1, 2, 0).reshape(64, NB)


def _host_prep(series, vars0, bias, Wx, Wh):
    series = np.asarray(series, dtype=np.float32)
    x2 = (series.astype(np.float64) ** 2).astype(BF16)
    wt = _weights_host(
        np.asarray(vars0, np.float32), np.asarray(bias, np.float32),
        np.asarray(Wx, np.float32), np.asarray(Wh, np.float32),
    )
    zero128 = np.zeros((128, 1), BF16)
    in_maps = []
    for i in range(NCORES):
        m = dict(wt)
        m["xa"] = np.ascontiguousarray(_pack_half(x2, i, 0))
        m["xb"] = np.ascontiguousarray(_pack_half(x2, i, 1))
        if i != 0:
            m["winit"] = zero128
        in_maps.append(m)
    return in_maps


def _assemble(results):
    hist = np.empty((T, K), dtype=np.float32)
    for i in range(NCORES):
        vout = results[i]["vout"].astype(np.float32)
        for h in range(2):
            for g in range(4):
                r = h * 4 + g
                reg = vout[:, r * NB:(r + 1) * NB]
                arr = reg.reshape(8, 16, NB).transpose(2, 0, 1).reshape(TT, 16)
                q0 = 0 if (i == 0 and h == 0) else W
                start = i * 65536 + h * HALF
                hist[start:start + HALF, 16 * g:16 * g + 16] = arr[q0:q0 + HALF]
    return hist


# ---------------------------------------------------------------------------
# numpy emulator
# ---------------------------------------------------------------------------

def emulate(inputs):
    in_maps = _host_prep(
        inputs["series"], inputs["vars0"], inputs["bias"],
        inputs["Wx"], inputs["Wh"],
    )
    results = []
    for m in in_maps:
        cb = m["constb"].astype(np.float32)
        lhsT_p = cb[:, 0:512]
        lhsT_dh = cb[0:64, 512:576]
        cf = m["constf"]
        bias_sb = cf[:, 0:4]
        whB = cf[:, 4].astype(np.float32)

        # phase 1
        d_all = np.empty((128, NB), np.float32)
        d_all[0:64] = lhsT_dh.T @ m["xa"].astype(np.float32)
        d_all[64:128] = lhsT_dh.T @ m["xb"].astype(np.float32)

        # scan: S_scan [128, 1+NB], col 0 = winit, col 1+b = S_b (bf16)
        S_scan = np.empty((128, 1 + NB), BF16)
        S_scan[:, 0] = m["winit"][:, 0]
        for c0 in range(0, NB, SC):
            c1 = min(c0 + SC, NB)
            st = S_scan[:, c0].astype(np.float32)
            for b in range(c0, c1):
                st = whB * st + d_all[:, b]
                S_scan[:, 1 + b] = st.astype(BF16)

        # tiles: rows 0..64 x2, rows 64..128 = S_{b-1} = S_scan cols 0..NB
        tiles = [np.zeros((128, NB), BF16), np.zeros((128, NB), BF16)]
        tiles[0][0:64] = m["xa"]
        tiles[1][0:64] = m["xb"]
        tiles[0][64:128] = S_scan[0:64, 0:NB]
        tiles[1][64:128] = S_scan[64:128, 0:NB]

        vout = np.empty((128, 8 * NB), BF16)
        for h in range(2):
            tf = tiles[h].astype(np.float32)
            for g in range(4):
                r = h * 4 + g
                ps = lhsT_p[:, 128 * g:128 * g + 128].T @ tf
                vout[:, r * NB:(r + 1) * NB] = (
                    ps + bias_sb[:, g:g + 1]
                ).astype(BF16)
        results.append({"vout": vout})
    return _assemble(results)


# ---------------------------------------------------------------------------
# Bass kernel
# ---------------------------------------------------------------------------

def _build_nc():
    import concourse.bacc as bacc
    import concourse.mybir as mybir
    import concourse.tile as tile

    f32 = mybir.dt.float32
    bf16 = mybir.dt.bfloat16

    nc = bacc.Bacc(None, target_bir_lowering=False)
    xa_d = nc.dram_tensor("xa", [64, NB], bf16, kind="ExternalInput")
    xb_d = nc.dram_tensor("xb", [64, NB], bf16, kind="ExternalInput")
    cb_d = nc.dram_tensor("constb", [128, 576], bf16, kind="ExternalInput")
    cf_d = nc.dram_tensor("constf", [128, 5], f32, kind="ExternalInput")
    wi_d = nc.dram_tensor("winit", [128, 1], bf16, kind="ExternalInput")
    vout_d = nc.dram_tensor("vout", [128, 8 * NB], bf16, kind="ExternalOutput")

    n_sc = (NB + SC - 1) // SC       # 9
    scan_eng = None

    with tile.TileContext(nc) as tc:
        with (
            tc.tile_pool(name="const", bufs=1) as cpool,
            tc.tile_pool(name="xbuf", bufs=1) as xpool,
            tc.tile_pool(name="sbuf_s", bufs=1) as spool,
            tc.tile_pool(name="stage", bufs=1) as stpool,
        ):
            scan_eng = nc.vector if SCAN_ENGINE == "vector" else nc.gpsimd
            # input tiles; staged pieces so phase 1 starts early
            tA = xpool.tile([128, NB], bf16)
            tB = xpool.tile([128, NB], bf16)
            pieces = [(0, 512), (512, 1024), (1024, 2048), (2048, 3072),
                      (3072, NB)]
            p0, p1 = pieces[0]
            nc.sync.dma_start(tA[0:64, p0:p1], xa_d[:, p0:p1])
            nc.sync.dma_start(tB[0:64, p0:p1], xb_d[:, p0:p1])

            cb_sb = cpool.tile([128, 576], bf16)
            nc.sync.dma_start(cb_sb[:], cb_d[:])
            cf_sb = cpool.tile([128, 5], f32)
            nc.sync.dma_start(cf_sb[:], cf_d[:])

            S_scan = spool.tile([128, 1 + NB], bf16)
            nc.sync.dma_start(S_scan[:, 0:1], wi_d[:])

            for p0, p1 in pieces[1:]:
                nc.sync.dma_start(tA[0:64, p0:p1], xa_d[:, p0:p1])
                nc.sync.dma_start(tB[0:64, p0:p1], xb_d[:, p0:p1])

            lhsT_p = cb_sb[:, 0:512]
            lhsT_dh = cb_sb[0:64, 512:576]
            bias_sb = cf_sb[:, 0:4]
            whB_col = cf_sb[:, 4:5]

            whB_wide = spool.tile([128, SCW], f32)
            nc.vector.memset(whB_wide[:], 1.0)
            nc.vector.tensor_scalar(
                whB_wide[:], whB_wide[:], whB_col, None,
                mybir.AluOpType.mult,
            )

            # ---- phase 1 + scans + sweeps share one PSUM scope ----

            stages = [stpool.tile([128, NB], bf16, tag=f"st{r}",
                                  name=f"stage{r}")
                      for r in range(8)]

            with (
                tc.tile_pool(name="dps", bufs=1, space="PSUM") as dps,
                tc.tile_pool(name="pps", bufs=1, space="PSUM") as pps,
            ):
                # d matmuls -> PSUM; scan reads PSUM directly.
                # Emit d-pair c, then its scan + carry; later d-pairs are
                # interleaved into sweep-0 emission (emit_more_d) so the PE
                # queue isn't blocked behind the scan-gated d-tag rotation.
                d_state = {"c": 0}

                def emit_d(c):
                    c0, c1 = c * SC, min((c + 1) * SC, NB)
                    n = c1 - c0
                    d_ps = dps.tile([128, SC], f32, tag=f"d{c % 2}",
                                    name=f"dpsx{c}")
                    nc.tensor.matmul(
                        d_ps[0:64, 0:n], lhsT_dh, tA[0:64, c0:c1],
                        start=True, stop=True, tile_position=(0, 0),
                    )
                    nc.tensor.matmul(
                        d_ps[64:128, 0:n], lhsT_dh, tB[0:64, c0:c1],
                        start=True, stop=True, tile_position=(0, 64),
                    )
                    scan_eng.tensor_tensor_scan(
                        S_scan[:, 1 + c0:1 + c1],
                        whB_wide[:, 0:n],
                        d_ps[:, 0:n],
                        S_scan[:, c0:c0 + 1],
                        mybir.AluOpType.mult,
                        mybir.AluOpType.add,
                    )
                    if c in (0, 1, 3, 5, 7, 8):
                        b0 = {0: 0, 1: 512, 3: 1024, 5: 2048, 7: 3072,
                              8: 4096}[c]
                        nc.scalar.dma_start(tA[64:128, b0:c1],
                                            S_scan[0:64, b0:c1])
                        nc.scalar.dma_start(tB[64:128, b0:c1],
                                            S_scan[64:128, b0:c1])

                def emit_more_d():
                    if d_state["c"] < n_sc:
                        emit_d(d_state["c"])
                        d_state["c"] += 1

                for _ in range(3):
                    emit_more_d()
                np_ps = 0
                for ci, (c0, c1) in enumerate(PCHUNKS):
                    for h in range(2):
                        th = tA if h == 0 else tB
                        for g in range(4):
                            emit_more_d()
                            r = h * 4 + g
                            p_ps = pps.tile([128, PC], f32,
                                            tag=f"p{np_ps % 3}")
                            # copy engine: Pool every 5th, else DVE/ACT
                            if np_ps % 5 in (0, 2):
                                ceng = "dve"
                            else:
                                ceng = "act"
                            np_ps += 1
                            done = 0
                            while done < c1 - c0:
                                n = min(512, c1 - c0 - done)
                                a0 = c0 + done
                                nc.tensor.matmul(
                                    p_ps[:, done:done + n],
                                    lhsT_p[:, 128 * g:128 * g + 128],
                                    th[:, a0:a0 + n],
                                    start=True, stop=True,
                                    tile_position=(0, 0),
                                )
                                done += n
                            # fused bias + bf16 cast
                            if ceng == "act":
                                nc.scalar.activation(
                                    stages[r][:, c0:c1], p_ps[:, 0:c1 - c0],
                                    mybir.ActivationFunctionType.Identity,
                                    bias=bias_sb[:, g:g + 1],
                                )
                            elif ceng == "dve":
                                nc.vector.tensor_scalar(
                                    stages[r][:, c0:c1], p_ps[:, 0:c1 - c0],
                                    1.0,
                                    bias_sb[:, g:g + 1],
                                    mybir.AluOpType.mult,
                                    mybir.AluOpType.add,
                                )
                            else:
                                nc.gpsimd.tensor_scalar(
                                    stages[r][:, c0:c1], p_ps[:, 0:c1 - c0],
                                    1.0,
                                    bias_sb[:, g:g + 1],
                                    mybir.AluOpType.mult,
                                    mybir.AluOpType.add,
                                )
                            # vout DMA per chunk, split across queues
                            eng = nc.gpsimd if np_ps % 2 == 0 else nc.sync
                            eng.dma_start(
                                vout_d[:, r * NB + c0:r * NB + c1],
                                stages[r][:, c0:c1],
                            )

    nc.compile()
    return nc


def run(inputs, trace=False, **kw):
    from concourse.bass_utils import run_bass_kernel_spmd

    if "nc" not in _CACHE:
        _CACHE["nc"] = _build_nc()
    nc = _CACHE["nc"]
    in_maps = _host_prep(
        inputs["series"], inputs["vars0"], inputs["bias"],
        inputs["Wx"], inputs["Wh"],
    )
    res = run_bass_kernel_spmd(
        nc, in_maps, core_ids=list(range(NCORES)), trace=trace, **kw
    )
    return _assemble(res.results), res


def kernel(series, vars0, bias, Wx, Wh):
    out, _ = run(
        {"series": series, "vars0": vars0, "bias": bias, "Wx": Wx, "Wh": Wh}
    )
    return out

